# revision 24
# baseline (speedup 1.0000x reference)
"""Trainium2 Bass kernel: CenterSurroundConvolution.

out[b,o,h,w] = sum_c center[b,c,h,w]*w_c[c,o] + surround[b,c,h,w]*w_s[c,o] + w_b[o]
where center = I[:,:,1:-1,1:-1], surround = (3x3 box sum of I) - center.

Rewritten as:  out = center @ (w_c - w_s) + box @ w_s + w_b
so only two channel-contraction matmuls are needed per pixel, and both
accumulate into the same PSUM bank.

Mapping (per NeuronCore, data-parallel over batch: 16 images / 8 cores):
  - Input is cast to bf16 on the host (halves input DMA; matmuls are bf16).
  - Each image is processed in horizontal bands (small first/last bands to
    shorten pipeline fill/drain); band input DMAs are prefetched ahead.
  - Vertical 3-tap sum: two row-shifted bf16 tensor_adds merged across the
    two 128-channel chunks (3D APs, 2x DVE mode).
  - Horizontal 3-tap sum: custom DVE op (SLIDE3P), one pass via two
    telescoping prefix scans over a front-zero-padded buffer:
       out[k] = scanA(P[k+2..]) - scanB(P[k..]) + P[k] = P[k]+P[k+1]+P[k+2]
    With P[0:2] == 0 this needs no seed scalars, so the GPSIMD seed chain
    of the earlier version is gone entirely.
  - Channel matmuls in bf16 (full-rate PE), K = 2x128 chunks, M = 2x128,
    N = 378/504 (3-4 output rows), 4 matmuls accumulating per PSUM bank.
  - ACT evacuates PSUM -> SBUF adding the bias, writing bf16 (halves the
    output DMA); the host casts back to fp32.
"""

import sys

import numpy as np

_TRN_REPO = "/opt/trn_rl_repo"
if _TRN_REPO not in sys.path:
    sys.path.insert(0, _TRN_REPO)

import concourse.bacc as bacc
import concourse.mybir as mybir
from concourse import bass_utils, tile
from concourse.dve_spec import AluOp, Spec, Src0, Src1, lower, scan
import concourse.dve_ops as dve_ops
from concourse.dve_uop import (
    DveOpSpec,
    UopConfig,
    UopDpConfig,
    AluInp,
    DelayInp,
    InpSel,
    OutPath,
    OutSel,
    Trigger,
)

# Problem shape (hardcoded per the task contract).
B, C_IN, C_OUT, H, W = 16, 256, 256, 128, 128
N_CORES = 8
IMG_PER_CORE = B // N_CORES          # 2
HO, WO = H - 2, W - 2                # 126, 126

# Variable band sizes: small first band (pipeline fill) and small last
# band (pipeline drain), 24-row bands in the middle.
BANDS = [3, 24, 24, 24, 24, 20, 7]   # output rows per band (sum = 126)
BAND_MAX = max(BANDS)
L_MAX = BAND_MAX * W                 # 3072
assert sum(BANDS) == HO
KC = C_IN // 128                     # 2 contraction chunks
MC = C_OUT // 128                    # 2 output-channel chunks
GP_ROWS = 12                         # t1 rows offloaded to GPSIMD (big bands)


def _chunks(rows):
    """Split band rows into matmul chunks of 4 (N=504) and 3 (N=378)."""
    n3 = 0
    while (rows - 3 * n3) % 4 != 0:
        n3 += 1
    return [4] * ((rows - 3 * n3) // 4) + [3] * n3


def _slide3p_ref(in0, in1, s0, s1, imm2):
    p = in0.shape[0]
    a0 = in0.reshape(p, -1)
    a1 = in1.reshape(p, -1)
    return (
        np.cumsum(a0, axis=1, dtype=np.float32)
        - np.cumsum(a1, axis=1, dtype=np.float32)
        + a1
    )


def _slide3_uops_2x():
    """Hand-written 2X_1PORT program for SLIDE3P_ANT.

    In 2x packed mode the engine delivers per cycle a0=A[2i] (SRC_0),
    a1=A[2i+1] (SRC_0_HI), b0=B[2i] (SRC_1), b1=B[2i+1] (SRC_1_HI) and
    writes a packed pair (WR0_LO, WR0_HI).

    Semantics (matching the 1x spec  out[k] = cumA[k] - cumB[k] + B[k]):
      keep one accumulator D = cumA - cumB over completed pairs; then
        D'       = D + (a0 + a1) - (b0 + b1)
        out_hi   = D' + b1
        out_lo   = out_hi - a1 + b0     (= D + a0)
    D stays the size of a 3-tap window value, so this is numerically
    tighter than differencing two large running sums.

    Input lanes (stock 2x convention): lane0=SRC_0 (read at stage 0 only),
    lane1=SRC_1 -> chain0, lane2=SRC_0_HI -> chain1, lane3=SRC_1_HI -> chain2.
    """
    E = 1
    # --- seed uop: zero the stage-3 accumulator flop, consume nothing ---
    seed = UopConfig()
    seed.enable_input(InpSel.ZERO, 1)            # chain0 = 0
    for s in range(3):
        seed.datapath_config[s].pass_through_delay(0)
    st3 = seed.datapath_config[3]
    st3.enable_alu(AluOp.BYPASS, AluInp.PREV_DELAY_0)
    seed.repeat_count = 1
    seed.trigger = (Trigger.COUNT, Trigger.NONE, Trigger.NONE)
    seed.next_uop = (1, 0, 0)

    # --- steady-state uop ---
    u = UopConfig()
    u.enable_input(InpSel.SRC_0, 0)              # a0 (stage-0 view)
    u.enable_input(InpSel.SRC_1, 1)              # b0 -> chain0
    u.enable_input(InpSel.SRC_0_HI, 2)           # a1 -> chain1
    u.enable_input(InpSel.SRC_1_HI, 3)           # b1 -> chain2
    u.require_inp0 = E
    u.require_inp1 = E
    u.trigger = (Trigger.SRC_TENSOR_DONE, Trigger.NONE, Trigger.NONE)
    u.next_uop = (0, 0, 0)
    dp = u.datapath_config
    # S0: SA = a0 + a1
    dp[0].enable_alu(AluOp.ADD, AluInp.PREV_ALU_OUT, AluInp.PREV_DELAY_1)
    dp[0].pass_through_delay(0, 1, 2)
    # S1: SB = b0 + b1 ; capture SA into chain3
    dp[1].enable_alu(AluOp.ADD, AluInp.PREV_DELAY_0, AluInp.PREV_DELAY_2)
    dp[1].pass_through_delay(0, 1, 2)
    dp[1].enable_delay_from_src(DelayInp.PREV_ALU_OUT, 3)
    # S2: T = SA - SB
    dp[2].enable_alu(AluOp.SUBTRACT, AluInp.PREV_DELAY_3, AluInp.PREV_ALU_OUT)
    dp[2].pass_through_delay(0, 1, 2)
    # S3: D += T   (accumulator flop)
    dp[3].enable_alu(AluOp.ADD, AluInp.CURR_ALU_OUT, AluInp.PREV_ALU_OUT)
    dp[3].pass_through_delay(0, 1, 2)
    # S4: OH = D + b1
    dp[4].enable_alu(AluOp.ADD, AluInp.PREV_ALU_OUT, AluInp.PREV_DELAY_2)
    dp[4].pass_through_delay(0, 1)
    # S5: U = OH - a1 ; capture OH into chain2
    dp[5].enable_alu(AluOp.SUBTRACT, AluInp.PREV_ALU_OUT, AluInp.PREV_DELAY_1)
    dp[5].pass_through_delay(0)
    dp[5].enable_delay_from_src(DelayInp.PREV_ALU_OUT, 2)
    # S6: OL = U + b0
    dp[6].enable_alu(AluOp.ADD, AluInp.PREV_ALU_OUT, AluInp.PREV_DELAY_0)
    dp[6].pass_through_delay(2)
    # S7: bypass OL; OH rides chain2 to the output mux
    dp[7].pass_through_alu()
    dp[7].pass_through_delay(2)
    u.enable_output(OutSel.ALU_OUT, OutPath.WR0_LO)
    u.enable_output(OutSel.DELAY_2, OutPath.WR0_HI)
    return [seed, u]


_SLIDE3 = None


def _get_slide3():
    """Register (once) the seedless 3-tap sliding-sum custom DVE op,
    with a hand-written 2X_1PORT perf-mode program."""
    global _SLIDE3
    if _SLIDE3 is not None:
        return _SLIDE3
    for op in dve_ops.OPS:
        if op.name == "SLIDE3P_ANT":
            _SLIDE3 = op
            return op
    body = scan(AluOp.ADD, Src0) - scan(AluOp.ADD, Src1) + Src1
    spec = Spec(body=body, reference=_slide3p_ref)
    uops2x = _slide3_uops_2x()
    shas = {}
    specs = {}
    for ver in ("v3", "v4"):
        tmp = DveOpSpec(
            name="SLIDE3P_ANT",
            uops=lower(spec, ver=ver),
            uops_2x=uops2x if ver == "v3" else None,
            rd1_en=True,
            perf_max=1 if ver == "v3" else 0,
        )
        shas[ver] = tmp.sha(ver)
        specs[ver] = tmp
    op = dve_ops.DveOp("SLIDE3P_ANT", spec, subdim=False, uops_sha=shas)
    dve_ops.OPS.append(op)
    dve_ops.CUSTOM_DVE_SPECS[op.name] = spec
    dve_ops._SUB_OPCODE_FOR_NAME[op.name] = dve_ops._CUSTOM_DVE_ROW_BASE + len(
        dve_ops.OPS
    ) - 1
    # Pre-seed the compile cache so the custom 2x table program (which
    # lower() cannot generate) is what table-gen writes.
    for ver, tmp in specs.items():
        tmp.opcode = dve_ops.get_dve_sub_opcode(op.name)
        dve_ops._COMPILE_CACHE[(op.name, ver)] = tmp
    _SLIDE3 = op
    return op


def build_module(n_img: int = IMG_PER_CORE):
    slide3 = _get_slide3()
    nc = bacc.Bacc(
        "TRN2", target_bir_lowering=False, debug=False, enable_asserts=False
    )
    f32 = mybir.dt.float32
    bf16 = mybir.dt.bfloat16

    I = nc.dram_tensor("I", [n_img, C_IN, H, W], bf16, kind="ExternalInput").ap()
    wcp = nc.dram_tensor("wcp", [C_IN, C_OUT], bf16, kind="ExternalInput").ap()
    ws = nc.dram_tensor("ws", [C_IN, C_OUT], bf16, kind="ExternalInput").ap()
    wb = nc.dram_tensor("wb", [C_OUT], f32, kind="ExternalInput").ap()
    out = nc.dram_tensor(
        "out", [n_img, C_OUT, HO, WO], bf16, kind="ExternalOutput"
    ).ap()

    with tile.TileContext(nc) as tc:
        with (
            tc.tile_pool(name="wts", bufs=1) as wpool,
            tc.tile_pool(name="io", bufs=5) as iopool,
            tc.tile_pool(name="rs", bufs=1) as rspool,
            tc.tile_pool(name="t1p", bufs=3) as t1pool,
            tc.tile_pool(name="box", bufs=3) as boxpool,
            tc.tile_pool(name="outp", bufs=2) as outpool,
            tc.tile_pool(name="ps", bufs=8, space="PSUM") as pspool,
        ):
            # Stationary weights: [128, w(2), k(2), m*128] (w=0: w_c - w_s, w=1: w_s)
            wt = wpool.tile([128, 2, KC, MC * 128], bf16)
            bias = wpool.tile([128, MC], f32)

            def emit_weight_dma():
                for wi, wsrc in enumerate((wcp, ws)):
                    for k in range(KC):
                        nc.sync.dma_start(
                            wt[:, wi, k, :], wsrc[k * 128 : (k + 1) * 128, :]
                        )
                nc.sync.dma_start(bias[:, :], wb.rearrange("(m p) -> p m", p=128))

            # Persistent scratch: P is the padded column-sum buffer consumed
            # by the slide3 scans; its first two elements per k stay zero
            # forever (seedless telescoping). Each band zeroes its own
            # 2-element tail, so no full-buffer memset is needed (which
            # would serialize ~6us ahead of the first DVE op).
            LP = L_MAX + 4
            P = rspool.tile([128, KC, LP], bf16, name="csP")
            nc.vector.memset(P[:, :, 0:2], 0.0)

            # Interleave the two images' bands so the small fill/drain bands
            # of one image always overlap the big bands of the other --
            # otherwise the PE starves at the image boundary while the DVE
            # rebuilds the next image's first box sums.
            per_img = []
            for b in range(n_img):
                h0 = 0
                row = []
                for band_rows in BANDS:
                    row.append((b, h0, band_rows))
                    h0 += band_rows
                per_img.append(row)
            jobs = [j for tup in zip(*per_img) for j in tup]

            def emit_dma(job):
                b, h0, band_rows = job
                l_in = (band_rows + 2) * W
                Ib = I[b].rearrange("c h w -> c (h w)")
                it = iopool.tile(
                    [128, KC, l_in], bf16, tag="it", name=f"it{b}_{h0}"
                )
                src = Ib.rearrange("(k p) x -> p k x", p=128)[
                    :, :, h0 * W : h0 * W + l_in
                ]
                nc.sync.dma_start(it[:, :, :], src)
                return it

            def emit_compute(job, it):
                b, h0, band_rows = job
                l_cs = band_rows * W
                Ob = out[b].rearrange("(m p) h w -> p m (h w)", p=128)
                it_rows = it.rearrange("p k (h w) -> p k h w", w=W)
                boxt = boxpool.tile(
                    [128, KC, l_cs + 2], bf16, tag="box", name="boxt"
                )
                # Vertical 3-tap sum, both k chunks in one op (3D APs).
                t1 = t1pool.tile([128, KC, l_cs], bf16, tag="t1", name="t1")
                nc.vector.tensor_add(
                    t1[:, :, 0:l_cs], it[:, :, 0:l_cs], it[:, :, 2 * W :]
                )
                nc.vector.tensor_add(
                    P[:, :, 2 : 2 + l_cs], t1[:, :, 0:l_cs], it[:, :, W : W + l_cs]
                )
                nc.vector.memset(P[:, :, 2 + l_cs : 4 + l_cs], 0.0)
                # Horizontal 3-tap sum via telescoping scans; box[j] lands at
                # boxt[..., j+2]. Both k chunks ride one op: the zeroed tail
                # and front pad make the k-boundary contribution cancel
                # exactly in the scan difference.
                bi = nc.vector._custom_dve(
                    slide3,
                    out=boxt[:, :, :],
                    in0=P[:, :, 2 : 4 + l_cs],
                    in1=P[:, :, 0 : 2 + l_cs],
                )
                bi.ins.perf_max = 1  # allow 2X_1PORT

                ot = outpool.tile(
                    [128, MC, band_rows * WO], bf16, tag="ot", name="ot"
                )
                box_rows = [
                    boxt[:, k, 2 : 2 + l_cs].rearrange("p (h w) -> p h w", w=W)
                    for k in range(KC)
                ]
                for m in range(MC):
                    r0 = 0
                    for crows in _chunks(band_rows):
                        nmm = crows * WO
                        ps = pspool.tile([128, 512], f32, tag="ps", name="ps")
                        quads = [(0, 0), (0, 1), (1, 0), (1, 1)]
                        for qi, (wi, k) in enumerate(quads):
                            lhsT = wt[:, wi, k, m * 128 : (m + 1) * 128]
                            if wi == 0:
                                rhs = it_rows[
                                    :, k, 1 + r0 : 1 + r0 + crows, 1 : 1 + WO
                                ]
                            else:
                                rhs = box_rows[k][:, r0 : r0 + crows, 0:WO]
                            nc.tensor.matmul(
                                ps[:, 0:nmm], lhsT, rhs,
                                start=(qi == 0), stop=(qi == 3),
                            )
                        nc.scalar.activation(
                            ot[:, m, r0 * WO : r0 * WO + nmm],
                            ps[:, 0:nmm],
                            mybir.ActivationFunctionType.Identity,
                            bias=bias[:, m : m + 1],
                        )
                        r0 += crows
                    # drain this m-half as soon as its last ACT finishes;
                    # issued from the ACT engine itself (in-order, no
                    # cross-engine semaphore, keeps the sync engine free
                    # for input prefetch)
                    nc.scalar.dma_start(
                        Ob[:, m, h0 * WO : (h0 + band_rows) * WO],
                        ot[:, m, :],
                    )

            # Input DMAs for the first bands go out before the (serially
            # issued) weight DMAs: the DVE needs band 0 well before the PE
            # needs the weights.
            PREFETCH = 4
            pending = []
            for j, job in enumerate(jobs):
                pending.append((job, emit_dma(job)))
                if j == 3:
                    emit_weight_dma()
                if len(pending) > PREFETCH:
                    pj, pit = pending.pop(0)
                    emit_compute(pj, pit)
            for pj, pit in pending:
                emit_compute(pj, pit)
    nc.finalize()
    return nc


_MODULE = None


def _get_module():
    global _MODULE
    if _MODULE is None:
        _MODULE = build_module()
    return _MODULE


def run(I, w_c, w_s, w_b, trace=False, **trace_kwargs):
    import ml_dtypes

    I = np.ascontiguousarray(
        np.asarray(I, dtype=np.float32).astype(ml_dtypes.bfloat16)
    )
    w_c = np.asarray(w_c, dtype=np.float32)
    w_s = np.asarray(w_s, dtype=np.float32)
    wcp = np.ascontiguousarray((w_c - w_s).astype(ml_dtypes.bfloat16))
    ws16 = np.ascontiguousarray(w_s.astype(ml_dtypes.bfloat16))
    wb = np.ascontiguousarray(np.asarray(w_b), dtype=np.float32)

    nc = _get_module()
    in_maps = [
        {
            "I": I[c * IMG_PER_CORE : (c + 1) * IMG_PER_CORE],
            "wcp": wcp,
            "ws": ws16,
            "wb": wb,
        }
        for c in range(N_CORES)
    ]
    res = bass_utils.run_bass_kernel_spmd(
        nc, in_maps, core_ids=list(range(N_CORES)), trace=trace, **trace_kwargs
    )
    out = np.concatenate(
        [np.asarray(r["out"], dtype=np.float32) for r in res.results], axis=0
    )
    return out, res


def kernel(I, w_c, w_s, w_b):
    out, _ = run(I, w_c, w_s, w_b)
    return out


if __name__ == "__main__":
    rng = np.random.default_rng(0)
    I = rng.standard_normal((B, C_IN, H, W), dtype=np.float32)
    w_c = rng.standard_normal((C_IN, C_OUT), dtype=np.float32) * 0.0625
    w_s = rng.standard_normal((C_IN, C_OUT), dtype=np.float32) * 0.0078
    w_b = np.zeros((C_OUT,), dtype=np.float32)
    o = kernel(I=I, w_c=w_c, w_s=w_s, w_b=w_b)
    print("out", o.shape, o.dtype, float(np.abs(o).mean()))


# revision 25
# speedup vs baseline: 1.0091x; 1.0091x over previous
"""Trainium2 Bass kernel: CenterSurroundConvolution.

out[b,o,h,w] = sum_c center[b,c,h,w]*w_c[c,o] + surround[b,c,h,w]*w_s[c,o] + w_b[o]
where center = I[:,:,1:-1,1:-1], surround = (3x3 box sum of I) - center.

Rewritten as:  out = center @ (w_c - w_s) + box @ w_s + w_b
so only two channel-contraction matmuls are needed per pixel, and both
accumulate into the same PSUM bank.

Mapping (per NeuronCore, data-parallel over batch: 16 images / 8 cores):
  - Input is cast to bf16 on the host (halves input DMA; matmuls are bf16).
  - Each image is processed in horizontal bands (small first/last bands to
    shorten pipeline fill/drain); band input DMAs are prefetched ahead.
  - Vertical 3-tap sum: two row-shifted bf16 tensor_adds merged across the
    two 128-channel chunks (3D APs, 2x DVE mode).
  - Horizontal 3-tap sum: custom DVE op (SLIDE3P), one pass via two
    telescoping prefix scans over a front-zero-padded buffer:
       out[k] = scanA(P[k+2..]) - scanB(P[k..]) + P[k] = P[k]+P[k+1]+P[k+2]
    With P[0:2] == 0 this needs no seed scalars, so the GPSIMD seed chain
    of the earlier version is gone entirely.
  - Channel matmuls in bf16 (full-rate PE), K = 2x128 chunks, M = 2x128,
    N = 378/504 (3-4 output rows), 4 matmuls accumulating per PSUM bank.
  - ACT evacuates PSUM -> SBUF adding the bias, writing bf16 (halves the
    output DMA); the host casts back to fp32.
"""

import sys

import numpy as np

_TRN_REPO = "/opt/trn_rl_repo"
if _TRN_REPO not in sys.path:
    sys.path.insert(0, _TRN_REPO)

import concourse.bacc as bacc
import concourse.mybir as mybir
from concourse import bass_utils, tile
from concourse.dve_spec import AluOp, Spec, Src0, Src1, lower, scan
import concourse.dve_ops as dve_ops
from concourse.dve_uop import (
    DveOpSpec,
    UopConfig,
    UopDpConfig,
    AluInp,
    DelayInp,
    InpSel,
    OutPath,
    OutSel,
    Trigger,
)

# Problem shape (hardcoded per the task contract).
B, C_IN, C_OUT, H, W = 16, 256, 256, 128, 128
N_CORES = 8
IMG_PER_CORE = B // N_CORES          # 2
HO, WO = H - 2, W - 2                # 126, 126

# Variable band sizes: small first band (pipeline fill) and small last
# band (pipeline drain), 24-row bands in the middle.
BANDS = [3, 24, 24, 24, 24, 20, 7]   # output rows per band (sum = 126)
BAND_MAX = max(BANDS)
L_MAX = BAND_MAX * W                 # 3072
assert sum(BANDS) == HO
KC = C_IN // 128                     # 2 contraction chunks
MC = C_OUT // 128                    # 2 output-channel chunks
GP_ROWS = 12                         # t1 rows offloaded to GPSIMD (big bands)


def _chunks(rows):
    """Split band rows into matmul chunks of 4 (N=504) and 3 (N=378)."""
    n3 = 0
    while (rows - 3 * n3) % 4 != 0:
        n3 += 1
    return [4] * ((rows - 3 * n3) // 4) + [3] * n3


def _slide3p_ref(in0, in1, s0, s1, imm2):
    p = in0.shape[0]
    a0 = in0.reshape(p, -1)
    a1 = in1.reshape(p, -1)
    return (
        np.cumsum(a0, axis=1, dtype=np.float32)
        - np.cumsum(a1, axis=1, dtype=np.float32)
        + a1
    )


def _slide3_uops_2x():
    """Hand-written 2X_1PORT program for SLIDE3P_ANT.

    In 2x packed mode the engine delivers per cycle a0=A[2i] (SRC_0),
    a1=A[2i+1] (SRC_0_HI), b0=B[2i] (SRC_1), b1=B[2i+1] (SRC_1_HI) and
    writes a packed pair (WR0_LO, WR0_HI).

    Semantics (matching the 1x spec  out[k] = cumA[k] - cumB[k] + B[k]):
      keep one accumulator D = cumA - cumB over completed pairs; then
        D'       = D + (a0 + a1) - (b0 + b1)
        out_hi   = D' + b1
        out_lo   = out_hi - a1 + b0     (= D + a0)
    D stays the size of a 3-tap window value, so this is numerically
    tighter than differencing two large running sums.

    Input lanes (stock 2x convention): lane0=SRC_0 (read at stage 0 only),
    lane1=SRC_1 -> chain0, lane2=SRC_0_HI -> chain1, lane3=SRC_1_HI -> chain2.
    """
    E = 1
    # --- seed uop: zero the stage-3 accumulator flop, consume nothing ---
    seed = UopConfig()
    seed.enable_input(InpSel.ZERO, 1)            # chain0 = 0
    for s in range(3):
        seed.datapath_config[s].pass_through_delay(0)
    st3 = seed.datapath_config[3]
    st3.enable_alu(AluOp.BYPASS, AluInp.PREV_DELAY_0)
    seed.repeat_count = 1
    seed.trigger = (Trigger.COUNT, Trigger.NONE, Trigger.NONE)
    seed.next_uop = (1, 0, 0)

    # --- steady-state uop ---
    u = UopConfig()
    u.enable_input(InpSel.SRC_0, 0)              # a0 (stage-0 view)
    u.enable_input(InpSel.SRC_1, 1)              # b0 -> chain0
    u.enable_input(InpSel.SRC_0_HI, 2)           # a1 -> chain1
    u.enable_input(InpSel.SRC_1_HI, 3)           # b1 -> chain2
    u.require_inp0 = E
    u.require_inp1 = E
    u.trigger = (Trigger.SRC_TENSOR_DONE, Trigger.NONE, Trigger.NONE)
    u.next_uop = (0, 0, 0)
    dp = u.datapath_config
    # S0: SA = a0 + a1
    dp[0].enable_alu(AluOp.ADD, AluInp.PREV_ALU_OUT, AluInp.PREV_DELAY_1)
    dp[0].pass_through_delay(0, 1, 2)
    # S1: SB = b0 + b1 ; capture SA into chain3
    dp[1].enable_alu(AluOp.ADD, AluInp.PREV_DELAY_0, AluInp.PREV_DELAY_2)
    dp[1].pass_through_delay(0, 1, 2)
    dp[1].enable_delay_from_src(DelayInp.PREV_ALU_OUT, 3)
    # S2: T = SA - SB
    dp[2].enable_alu(AluOp.SUBTRACT, AluInp.PREV_DELAY_3, AluInp.PREV_ALU_OUT)
    dp[2].pass_through_delay(0, 1, 2)
    # S3: D += T   (accumulator flop)
    dp[3].enable_alu(AluOp.ADD, AluInp.CURR_ALU_OUT, AluInp.PREV_ALU_OUT)
    dp[3].pass_through_delay(0, 1, 2)
    # S4: OH = D + b1
    dp[4].enable_alu(AluOp.ADD, AluInp.PREV_ALU_OUT, AluInp.PREV_DELAY_2)
    dp[4].pass_through_delay(0, 1)
    # S5: U = OH - a1 ; capture OH into chain2
    dp[5].enable_alu(AluOp.SUBTRACT, AluInp.PREV_ALU_OUT, AluInp.PREV_DELAY_1)
    dp[5].pass_through_delay(0)
    dp[5].enable_delay_from_src(DelayInp.PREV_ALU_OUT, 2)
    # S6: OL = U + b0
    dp[6].enable_alu(AluOp.ADD, AluInp.PREV_ALU_OUT, AluInp.PREV_DELAY_0)
    dp[6].pass_through_delay(2)
    # S7: bypass OL; OH rides chain2 to the output mux
    dp[7].pass_through_alu()
    dp[7].pass_through_delay(2)
    u.enable_output(OutSel.ALU_OUT, OutPath.WR0_LO)
    u.enable_output(OutSel.DELAY_2, OutPath.WR0_HI)
    return [seed, u]


_SLIDE3 = None


def _get_slide3():
    """Register (once) the seedless 3-tap sliding-sum custom DVE op,
    with a hand-written 2X_1PORT perf-mode program."""
    global _SLIDE3
    if _SLIDE3 is not None:
        return _SLIDE3
    for op in dve_ops.OPS:
        if op.name == "SLIDE3P_ANT":
            _SLIDE3 = op
            return op
    body = scan(AluOp.ADD, Src0) - scan(AluOp.ADD, Src1) + Src1
    spec = Spec(body=body, reference=_slide3p_ref)
    uops2x = _slide3_uops_2x()
    shas = {}
    specs = {}
    for ver in ("v3", "v4"):
        tmp = DveOpSpec(
            name="SLIDE3P_ANT",
            uops=lower(spec, ver=ver),
            uops_2x=uops2x if ver == "v3" else None,
            rd1_en=True,
            perf_max=1 if ver == "v3" else 0,
        )
        shas[ver] = tmp.sha(ver)
        specs[ver] = tmp
    op = dve_ops.DveOp("SLIDE3P_ANT", spec, subdim=False, uops_sha=shas)
    dve_ops.OPS.append(op)
    dve_ops.CUSTOM_DVE_SPECS[op.name] = spec
    dve_ops._SUB_OPCODE_FOR_NAME[op.name] = dve_ops._CUSTOM_DVE_ROW_BASE + len(
        dve_ops.OPS
    ) - 1
    # Pre-seed the compile cache so the custom 2x table program (which
    # lower() cannot generate) is what table-gen writes.
    for ver, tmp in specs.items():
        tmp.opcode = dve_ops.get_dve_sub_opcode(op.name)
        dve_ops._COMPILE_CACHE[(op.name, ver)] = tmp
    _SLIDE3 = op
    return op


def build_module(n_img: int = IMG_PER_CORE):
    slide3 = _get_slide3()
    nc = bacc.Bacc(
        "TRN2", target_bir_lowering=False, debug=False, enable_asserts=False
    )
    f32 = mybir.dt.float32
    bf16 = mybir.dt.bfloat16

    I = nc.dram_tensor("I", [n_img, C_IN, H, W], bf16, kind="ExternalInput").ap()
    wcp = nc.dram_tensor("wcp", [C_IN, C_OUT], bf16, kind="ExternalInput").ap()
    ws = nc.dram_tensor("ws", [C_IN, C_OUT], bf16, kind="ExternalInput").ap()
    wb = nc.dram_tensor("wb", [C_OUT], f32, kind="ExternalInput").ap()
    out = nc.dram_tensor(
        "out", [n_img, C_OUT, HO, WO], bf16, kind="ExternalOutput"
    ).ap()

    with tile.TileContext(nc) as tc:
        with (
            tc.tile_pool(name="wts", bufs=1) as wpool,
            tc.tile_pool(name="io", bufs=5) as iopool,
            tc.tile_pool(name="rs", bufs=1) as rspool,
            tc.tile_pool(name="t1p", bufs=3) as t1pool,
            tc.tile_pool(name="box", bufs=3) as boxpool,
            tc.tile_pool(name="outp", bufs=2) as outpool,
            tc.tile_pool(name="ps", bufs=8, space="PSUM") as pspool,
        ):
            # Stationary weights: [128, w(2), k(2), m*128] (w=0: w_c - w_s, w=1: w_s)
            wt = wpool.tile([128, 2, KC, MC * 128], bf16)
            bias = wpool.tile([128, MC], f32)

            def emit_weight_dma():
                for wi, wsrc in enumerate((wcp, ws)):
                    for k in range(KC):
                        nc.sync.dma_start(
                            wt[:, wi, k, :], wsrc[k * 128 : (k + 1) * 128, :]
                        )
                nc.sync.dma_start(bias[:, :], wb.rearrange("(m p) -> p m", p=128))

            # Persistent scratch: P is the padded column-sum buffer consumed
            # by the slide3 scans; its first two elements per k stay zero
            # forever (seedless telescoping). Each band zeroes its own
            # 2-element tail, so no full-buffer memset is needed (which
            # would serialize ~6us ahead of the first DVE op).
            LP = L_MAX + 4
            P = rspool.tile([128, KC, LP], bf16, name="csP")
            nc.vector.memset(P[:, :, 0:2], 0.0)

            # Interleave the two images' bands so the small fill/drain bands
            # of one image always overlap the big bands of the other --
            # otherwise the PE starves at the image boundary while the DVE
            # rebuilds the next image's first box sums.
            per_img = []
            for b in range(n_img):
                h0 = 0
                row = []
                for band_rows in BANDS:
                    row.append((b, h0, band_rows))
                    h0 += band_rows
                per_img.append(row)
            jobs = [j for tup in zip(*per_img) for j in tup]

            def emit_dma(job):
                b, h0, band_rows = job
                l_in = (band_rows + 2) * W
                Ib = I[b].rearrange("c h w -> c (h w)")
                it = iopool.tile(
                    [128, KC, l_in], bf16, tag="it", name=f"it{b}_{h0}"
                )
                src = Ib.rearrange("(k p) x -> p k x", p=128)[
                    :, :, h0 * W : h0 * W + l_in
                ]
                nc.sync.dma_start(it[:, :, :], src)
                return it

            def emit_compute(job, it):
                b, h0, band_rows = job
                l_cs = band_rows * W
                Ob = out[b].rearrange("(m p) h w -> p m (h w)", p=128)
                it_rows = it.rearrange("p k (h w) -> p k h w", w=W)
                boxt = boxpool.tile(
                    [128, KC, l_cs + 2], bf16, tag="box", name="boxt"
                )
                # Vertical 3-tap sum, both k chunks in one op (3D APs).
                t1 = t1pool.tile([128, KC, l_cs], bf16, tag="t1", name="t1")
                nc.vector.tensor_add(
                    t1[:, :, 0:l_cs], it[:, :, 0:l_cs], it[:, :, 2 * W :]
                )
                nc.vector.tensor_add(
                    P[:, :, 2 : 2 + l_cs], t1[:, :, 0:l_cs], it[:, :, W : W + l_cs]
                )
                nc.vector.memset(P[:, :, 2 + l_cs : 4 + l_cs], 0.0)
                # Horizontal 3-tap sum via telescoping scans; box[j] lands at
                # boxt[..., j+2]. Both k chunks ride one op: the zeroed tail
                # and front pad make the k-boundary contribution cancel
                # exactly in the scan difference.
                bi = nc.vector._custom_dve(
                    slide3,
                    out=boxt[:, :, :],
                    in0=P[:, :, 2 : 4 + l_cs],
                    in1=P[:, :, 0 : 2 + l_cs],
                )
                bi.ins.perf_max = 1  # allow 2X_1PORT

                ot = outpool.tile(
                    [128, MC, band_rows * WO], bf16, tag="ot", name="ot"
                )
                box_rows = [
                    boxt[:, k, 2 : 2 + l_cs].rearrange("p (h w) -> p h w", w=W)
                    for k in range(KC)
                ]
                for m in range(MC):
                    r0 = 0
                    for crows in _chunks(band_rows):
                        nmm = crows * WO
                        ps = pspool.tile([128, 512], f32, tag="ps", name="ps")
                        quads = [(0, 0), (0, 1), (1, 0), (1, 1)]
                        for qi, (wi, k) in enumerate(quads):
                            lhsT = wt[:, wi, k, m * 128 : (m + 1) * 128]
                            if wi == 0:
                                rhs = it_rows[
                                    :, k, 1 + r0 : 1 + r0 + crows, 1 : 1 + WO
                                ]
                            else:
                                rhs = box_rows[k][:, r0 : r0 + crows, 0:WO]
                            nc.tensor.matmul(
                                ps[:, 0:nmm], lhsT, rhs,
                                start=(qi == 0), stop=(qi == 3),
                            )
                        nc.scalar.activation(
                            ot[:, m, r0 * WO : r0 * WO + nmm],
                            ps[:, 0:nmm],
                            mybir.ActivationFunctionType.Identity,
                            bias=bias[:, m : m + 1],
                        )
                        r0 += crows
                    # drain this m-half as soon as its last ACT finishes;
                    # issued from the ACT engine itself (in-order, no
                    # cross-engine semaphore, keeps the sync engine free
                    # for input prefetch)
                    nc.scalar.dma_start(
                        Ob[:, m, h0 * WO : (h0 + band_rows) * WO],
                        ot[:, m, :],
                    )

            # Input DMAs for the first bands go out before the (serially
            # issued) weight DMAs: the DVE needs band 0 well before the PE
            # needs the weights.
            PREFETCH = 4
            pending = []
            for j, job in enumerate(jobs):
                pending.append((job, emit_dma(job)))
                if j == 1:
                    emit_weight_dma()
                if len(pending) > PREFETCH:
                    pj, pit = pending.pop(0)
                    emit_compute(pj, pit)
            for pj, pit in pending:
                emit_compute(pj, pit)
    nc.finalize()
    return nc


_MODULE = None


def _get_module():
    global _MODULE
    if _MODULE is None:
        _MODULE = build_module()
    return _MODULE


def run(I, w_c, w_s, w_b, trace=False, **trace_kwargs):
    import ml_dtypes

    I = np.ascontiguousarray(
        np.asarray(I, dtype=np.float32).astype(ml_dtypes.bfloat16)
    )
    w_c = np.asarray(w_c, dtype=np.float32)
    w_s = np.asarray(w_s, dtype=np.float32)
    wcp = np.ascontiguousarray((w_c - w_s).astype(ml_dtypes.bfloat16))
    ws16 = np.ascontiguousarray(w_s.astype(ml_dtypes.bfloat16))
    wb = np.ascontiguousarray(np.asarray(w_b), dtype=np.float32)

    nc = _get_module()
    in_maps = [
        {
            "I": I[c * IMG_PER_CORE : (c + 1) * IMG_PER_CORE],
            "wcp": wcp,
            "ws": ws16,
            "wb": wb,
        }
        for c in range(N_CORES)
    ]
    res = bass_utils.run_bass_kernel_spmd(
        nc, in_maps, core_ids=list(range(N_CORES)), trace=trace, **trace_kwargs
    )
    out = np.concatenate(
        [np.asarray(r["out"], dtype=np.float32) for r in res.results], axis=0
    )
    return out, res


def kernel(I, w_c, w_s, w_b):
    out, _ = run(I, w_c, w_s, w_b)
    return out


if __name__ == "__main__":
    rng = np.random.default_rng(0)
    I = rng.standard_normal((B, C_IN, H, W), dtype=np.float32)
    w_c = rng.standard_normal((C_IN, C_OUT), dtype=np.float32) * 0.0625
    w_s = rng.standard_normal((C_IN, C_OUT), dtype=np.float32) * 0.0078
    w_b = np.zeros((C_OUT,), dtype=np.float32)
    o = kernel(I=I, w_c=w_c, w_s=w_s, w_b=w_b)
    print("out", o.shape, o.dtype, float(np.abs(o).mean()))
